# revision 1
# baseline (speedup 1.0000x reference)
"""Multi-head graph attention (GAT-style) Trainium2 kernel.

Problem: out[b,h,i,o] = softmax_j(mask(leakyrelu_0.2(src[b,h,i] + dst[b,h,j])))
         @ h_prime[b,h,:,:] + bias
with h_prime = h @ w[h], src/dst = tanh(h_prime) @ a_src/a_dst.

Strategy: pure data-parallel over the 512-graph batch across 8 NeuronCores
(64 graphs per core, no collectives). Per graph, everything is built with PE
matmuls in a transposed-attention layout so no on-chip transposes and no
DVE broadcast passes are needed:

  - h_primeT (per head-pair) and h_prime (natural) via bf16 matmuls.
  - src/dst coefficient rows via tiny matmuls against constant A matrices.
  - logitsT[j, h*128+i] = dst_h[j] + src_h[i] via one K=9 f32r matmul
    (8 dst rows + ones row) against [blockind ; src_flat], plus the additive
    adjacency mask (0 / -250, exact in bf16) via an identity matmul that
    accumulates into the same PSUM tile.
  - leaky relu = ACT Prelu(alpha=0.2) (verified exact on HW), exp on ACT.
  - row sums ride the final matmul as a ones column appended to h_prime;
    normalization = DVE reciprocal + one broadcast multiply.
"""

import numpy as np

BS, N, HEADS, DIN, DOUT = 512, 128, 8, 64, 64
NCORES = 8
BSH = BS // NCORES  # graphs per core
MASKVAL = -250.0

_cache = {}


def _build_nc():
    import concourse.bass as bass
    import concourse.mybir as mybir
    import concourse.tile as tile

    f32 = mybir.dt.float32
    f16 = mybir.dt.float16
    bf16 = mybir.dt.bfloat16
    AF = mybir.ActivationFunctionType

    nc = bass.Bass("TRN2", target_bir_lowering=False, debug=False)

    # DRAM inputs (per core)
    hT_d = nc.dram_tensor("hT", [BSH, DIN, N], bf16, kind="ExternalInput").ap()
    adjmT_d = nc.dram_tensor("adjmT", [BSH, N, N], bf16, kind="ExternalInput").ap()
    w_all_d = nc.dram_tensor("w_all", [DIN, HEADS * DOUT], bf16, kind="ExternalInput").ap()
    # a_mats: per pair p, cols [16p:16p+8] = zero-padded dst cols (head h nonzero
    # only if h//2==p, rows q*64:(q+1)*64), cols [16p+8:16p+16] = same for src.
    a_mats_d = nc.dram_tensor("a_mats", [128, 64], bf16, kind="ExternalInput").ap()
    blockind_d = nc.dram_tensor("blockind", [HEADS, HEADS * N], f16, kind="ExternalInput").ap()
    ident_d = nc.dram_tensor("ident", [128, 128], bf16, kind="ExternalInput").ap()
    # DRAM output: out[b, i, h*64+o]
    out_d = nc.dram_tensor("out", [BSH, N, HEADS * DOUT], f32, kind="ExternalOutput").ap()

    with tile.TileContext(nc) as tc:
        with (
            tc.tile_pool(name="consts", bufs=1) as cpool,
            tc.tile_pool(name="inbuf", bufs=4) as inpool,
            tc.tile_pool(name="mid", bufs=3) as midpool,
            tc.tile_pool(name="attn", bufs=2) as attnpool,
            tc.tile_pool(name="outbuf", bufs=3) as outpool,
            # PSUM bank budget (8): hpT 2 + hp 1 + S2b 1 + at 2 + oa 2
            tc.tile_pool(name="ps_big", bufs=1, space="PSUM") as psbig,
            tc.tile_pool(name="ps_s", bufs=1, space="PSUM") as pss,
            tc.tile_pool(name="ps_at", bufs=2, space="PSUM") as psat,
            tc.tile_pool(name="ps_out", bufs=2, space="PSUM") as psout,
        ):
            # ---- constants (loaded once) ----
            w_all = cpool.tile([DIN, HEADS * DOUT], bf16, tag="w_all")
            nc.sync.dma_start(w_all[:], w_all_d[:])
            a_mats = cpool.tile([128, 64], bf16, tag="a_mats")
            nc.sync.dma_start(a_mats[:], a_mats_d[:])
            ident = cpool.tile([128, 128], bf16, tag="ident")
            nc.sync.dma_start(ident[:], ident_d[:])

            for b0 in range(0, BSH, 2):
                # ---- inputs for this graph pair ----
                # hT_2b [i, b*128+n]
                hT_t = inpool.tile([DIN, 2 * N], bf16, tag="hT")
                nc.sync.dma_start(
                    hT_t[:], hT_d[b0 : b0 + 2].rearrange("b i n -> i b n")
                )
                adjT_t = inpool.tile([N, 2 * N], bf16, tag="adjT")
                nc.sync.dma_start(
                    adjT_t[:], adjmT_d[b0 : b0 + 2].rearrange("b j i -> j b i")
                )

                # ---- h_primeT per head-pair (both graphs):
                # hpT_2b[q*64+o, p*256 + b*128 + n]
                hpT_ps = psbig.tile([128, 1024], f32, tag="hpT")
                for p in range(4):
                    nc.tensor.matmul(
                        hpT_ps[:, p * 256 : (p + 1) * 256],
                        lhsT=w_all[:, p * 128 : (p + 1) * 128],
                        rhs=hT_t[:],
                        start=True,
                        stop=True,
                    )

                # tanh -> tT (bf16), same layout
                tT_t = midpool.tile([128, 1024], bf16, tag="tT")
                nc.scalar.activation(tT_t[:], hpT_ps[:], AF.Tanh)

                # ---- src/dst rows for both graphs: S_2b[h, k*256 + b*128 + n]
                # (k=0: dst, k=1: src) — each matmul output is contiguous.
                S_ps = pss.tile([HEADS, 512], f32, tag="S2b")
                for p in range(4):
                    nc.tensor.matmul(
                        S_ps[:, 0:256],
                        lhsT=a_mats[:, 16 * p : 16 * p + 8],
                        rhs=tT_t[:, p * 256 : (p + 1) * 256],
                        start=(p == 0),
                        stop=(p == 3),
                    )
                for p in range(4):
                    nc.tensor.matmul(
                        S_ps[:, 256:512],
                        lhsT=a_mats[:, 16 * p + 8 : 16 * p + 16],
                        rhs=tT_t[:, p * 256 : (p + 1) * 256],
                        start=(p == 0),
                        stop=(p == 3),
                    )

                for q in range(2):
                    b = b0 + q
                    # ---- per-graph tiles ----
                    bi_t = inpool.tile([HEADS + 1, HEADS * N], f16, tag="bi")
                    nc.sync.dma_start(bi_t[0:HEADS, :], blockind_d[:])

                    # h_prime natural [n, h*64+o]
                    hp_ps = psbig.tile([128, 512], f32, tag="hp")
                    nc.tensor.matmul(
                        hp_ps[:], lhsT=hT_t[:, q * N : (q + 1) * N], rhs=w_all[:],
                        start=True, stop=True,
                    )
                    # hp_aug [n, h*65 + o], col 64 of each head = 1.0
                    hpa_t = midpool.tile([128, HEADS * (DOUT + 1)], bf16, tag="hpa")
                    hpa_v = hpa_t[:].rearrange("p (h c) -> p h c", c=DOUT + 1)
                    nc.gpsimd.memset(hpa_v[:, :, DOUT], 1.0)
                    nc.vector.tensor_copy(
                        hpa_v[:, :, 0:DOUT],
                        hp_ps[:].rearrange("p (h c) -> p h c", c=DOUT),
                    )

                    # lhsT for the K=9 logits matmul: rows 0-7 = dst, row 8 = ones.
                    S_sb = midpool.tile([HEADS + 1, N], f16, tag="S_sb")
                    nc.gpsimd.memset(S_sb[:], 1.0)
                    nc.vector.tensor_copy(S_sb[0:HEADS, :], S_ps[:, q * N : (q + 1) * N])

                    # src rows -> SBUF, flatten [8,128] -> [1,1024] into bi_t[8]
                    Ss_sb = midpool.tile([HEADS, N], f16, tag="Ss_sb")
                    nc.vector.tensor_copy(Ss_sb[:], S_ps[:, 256 + q * N : 256 + (q + 1) * N])
                    nc.sync.dma_start(
                        bi_t[HEADS : HEADS + 1, :].rearrange("p (h n) -> p h n", n=N),
                        Ss_sb[:],
                    )

                    # ---- attention logits (transposed): AT[j, h*128+i] ----
                    AT_lo = psat.tile([128, 512], f32, tag="at")
                    AT_hi = psat.tile([128, 512], f32, tag="at")
                    lhsT9 = S_sb[0:9, :]
                    adj_q = adjT_t[:, q * N : (q + 1) * N]
                    adj_rep = adj_q.unsqueeze(1).broadcast_to([N, 4, N])
                    nc.tensor.matmul(AT_lo[:], lhsT=lhsT9, rhs=bi_t[:, 0:512],
                                     start=True, stop=False)
                    nc.tensor.matmul(AT_hi[:], lhsT=lhsT9, rhs=bi_t[:, 512:1024],
                                     start=True, stop=False)
                    nc.tensor.matmul(AT_lo[:], lhsT=ident[:], rhs=adj_rep,
                                     start=False, stop=True)
                    nc.tensor.matmul(AT_hi[:], lhsT=ident[:], rhs=adj_rep,
                                     start=False, stop=True)

                    # ---- leaky relu (exact via Prelu alpha=0.2), then exp ----
                    LR_t = attnpool.tile([128, 1024], f32, tag="LR")
                    nc.scalar.activation(LR_t[:, 0:512], AT_lo[:], AF.Prelu, alpha=0.2)
                    nc.scalar.activation(LR_t[:, 512:1024], AT_hi[:], AF.Prelu, alpha=0.2)
                    E_t = attnpool.tile([128, 1024], bf16, tag="E")
                    nc.scalar.activation(E_t[:], LR_t[:], AF.Exp)

                    # ---- numerator + row sums: out_aug[i, h*65+o], col64 = s_i ----
                    oa_lo = psout.tile([128, 4 * (DOUT + 1)], f32, tag="oa")
                    oa_hi = psout.tile([128, 4 * (DOUT + 1)], f32, tag="oa")
                    for h in range(HEADS):
                        oa = oa_lo if h < 4 else oa_hi
                        c0 = (h % 4) * (DOUT + 1)
                        nc.tensor.matmul(
                            oa[:, c0 : c0 + DOUT + 1],
                            lhsT=E_t[:, h * 128 : (h + 1) * 128],
                            rhs=hpa_t[:, h * (DOUT + 1) : (h + 1) * (DOUT + 1)],
                            start=True,
                            stop=True,
                        )

                    # ---- normalize: out = numerator / s ----
                    r_lo = outpool.tile([128, 4], f32, tag="rlo")
                    r_hi = outpool.tile([128, 4], f32, tag="rhi")
                    oa_lo_v = oa_lo[:].rearrange("p (h c) -> p h c", c=DOUT + 1)
                    oa_hi_v = oa_hi[:].rearrange("p (h c) -> p h c", c=DOUT + 1)
                    nc.vector.reciprocal(r_lo[:], oa_lo_v[:, :, DOUT])
                    nc.vector.reciprocal(r_hi[:], oa_hi_v[:, :, DOUT])

                    out_sb = outpool.tile([128, HEADS * DOUT], f32, tag="out_sb")
                    out_v = out_sb[:].rearrange("p (h c) -> p h c", c=DOUT)
                    nc.vector.tensor_mul(
                        out_v[:, 0:4, :], oa_lo_v[:, :, 0:DOUT],
                        r_lo[:].unsqueeze(2).broadcast_to([128, 4, DOUT]),
                    )
                    nc.vector.tensor_mul(
                        out_v[:, 4:8, :], oa_hi_v[:, :, 0:DOUT],
                        r_hi[:].unsqueeze(2).broadcast_to([128, 4, DOUT]),
                    )

                    nc.sync.dma_start(out_d[b], out_sb[:])

    _split_excess_waits(nc)
    return nc


def _split_excess_waits(nc, cap=1):
    """Walrus codegen accepts at most `cap` sync-wait commands per
    instruction; hoist excess waits onto standalone drains inserted before."""
    import concourse.mybir as mybir

    n_new = 0
    for _bbname, bbw in nc.bb_map.items():
        inner = bbw.bb
        il = list(inner.instructions)
        out, changed = [], False
        for inst in il:
            si = inst.sync_info
            waits = list(si.on_wait) if si and si.on_wait else []
            if len(waits) > cap:
                extra = waits[:-cap]
                for ci in range(0, len(extra), cap):
                    chunk = extra[ci : ci + cap]
                    nop = mybir.InstDrain(
                        name=f"{inst.name}_wsplit{ci}", ins=[], outs=[],
                        bass_is_fusable=False,
                    )
                    nop.engine = inst.engine
                    nop.sync_info = mybir.SyncInfo(on_wait=chunk, on_update=[])
                    nc.register_instruction(nop)
                    out.append(nop)
                    n_new += 1
                si.on_wait = waits[-cap:]
                changed = True
            out.append(inst)
        if changed:
            inner.instructions = out
    return n_new


def _host_prep(h, adj, w, a_src, a_dst):
    import ml_dtypes

    bf = ml_dtypes.bfloat16
    hT = np.ascontiguousarray(h.transpose(0, 2, 1)).astype(bf)  # [BS, DIN, N]
    adjmT = np.where(adj, 0.0, MASKVAL).astype(np.float32)
    adjmT = np.ascontiguousarray(adjmT.transpose(0, 2, 1)).astype(bf)  # [BS, j, i]
    w_all = np.ascontiguousarray(w.transpose(1, 0, 2).reshape(DIN, HEADS * DOUT)).astype(bf)
    a_mats = np.zeros((128, 64), np.float32)
    for p in range(4):
        for q in range(2):
            hh = 2 * p + q
            a_mats[q * 64 : (q + 1) * 64, 16 * p + hh] = a_dst[hh, :, 0]
            a_mats[q * 64 : (q + 1) * 64, 16 * p + 8 + hh] = a_src[hh, :, 0]
    a_mats = a_mats.astype(bf)
    blockind = np.zeros((HEADS, HEADS * N), np.float16)
    for k in range(HEADS):
        blockind[k, k * N : (k + 1) * N] = 1.0
    ident = np.eye(128, dtype=np.float32).astype(bf)
    return hT, adjmT, w_all, a_mats, blockind, ident


def _make_in_maps(h, adj, w, a_src, a_dst):
    hT, adjmT, w_all, a_mats, blockind, ident = _host_prep(h, adj, w, a_src, a_dst)
    in_maps = []
    for c in range(NCORES):
        sl = slice(c * BSH, (c + 1) * BSH)
        in_maps.append(
            {
                "hT": np.ascontiguousarray(hT[sl]),
                "adjmT": np.ascontiguousarray(adjmT[sl]),
                "w_all": w_all,
                "a_mats": a_mats,
                "blockind": blockind,
                "ident": ident,
            }
        )
    return in_maps


def _gather(results, bias):
    # results[c]["out"]: [BSH, N, HEADS*DOUT]
    full = np.concatenate([results[c]["out"] for c in range(NCORES)], axis=0)
    out = full.reshape(BS, N, HEADS, DOUT).transpose(0, 2, 1, 3)
    return np.ascontiguousarray(out + bias[None, None, None, :]).astype(np.float32)


def kernel(h, adj, w, a_src, a_dst, bias, _trace=False):
    from concourse.bass_utils import run_bass_kernel_spmd

    h = np.asarray(h, np.float32)
    adj = np.asarray(adj, bool)
    w = np.asarray(w, np.float32)
    a_src = np.asarray(a_src, np.float32)
    a_dst = np.asarray(a_dst, np.float32)
    bias = np.asarray(bias, np.float32)

    if "nc" not in _cache:
        _cache["nc"] = _build_nc()
    nc = _cache["nc"]

    in_maps = _make_in_maps(h, adj, w, a_src, a_dst)
    res = run_bass_kernel_spmd(nc, in_maps, core_ids=list(range(NCORES)), trace=_trace)
    out = _gather(res.results, bias)
    if _trace:
        _cache["last_result"] = res
    return out

